# revision 1
# baseline (speedup 1.0000x reference)
"""Trainium2 Bass kernel for single-token multi-head self-attention.

Problem (hardcoded):
  q: (1, 32, 512) f32, k/v: (8192, 32, 512) f32, 8 heads x 64 dim,
  scores = (q.k)/8, softcapped 10*tanh(.), softmax over klen, out = w.v.

Strategy: data-parallel over batch, 4 batches per core on 8 cores. The
problem is HBM-bandwidth bound, so K/V/q are staged to device HBM as fp16
(half the traffic; scores only need ~1e-2 accuracy headroom and the final
softmax renormalizes common-mode error away). Per core, K/V stream in
j-chunks of J_FOLD*128 rows (fp16 SBUF tiles (128, J_FOLD*2048)):
  - scores via DVE: k_t *= q_broadcast (fp16 2x mode), two fp16 tree-halving
    adds over d (2x mode), then fp32 segmented reduce of the remaining 16
  - softcap+exp on ACT: e = exp(10*tanh(scores/8)) -> fp16 (no max pass
    needed: scores are clipped to +-10 so exp(s) <= 2.2e4 fits fp16/fp32)
  - P@V and sum(e) accumulated on PE into fp32 PSUM across all chunks
    (lhsT = e-slice (128,8), rhs = v-slice (128,512); ones column for the
    softmax denominator)
Epilogue ships the raw PV block (8, 4x512) and exp-sums (32,) to DRAM in
fp32; the tiny diagonal extraction out[b,h,:] = pv[h, b, h*64:] / s[b,h]
is done on the host (64 KB per core, negligible).
"""

import numpy as np

import concourse.bass as bass
import concourse.bacc as bacc
import concourse.tile as tile
from concourse import mybir
from concourse.bass_utils import run_bass_kernel_spmd

N_CORES = 8
KLEN = 8192
BSZ = 32
D_MODEL = 512
N_HEAD = 8
D_HEAD = 64
B_PER_CORE = BSZ // N_CORES            # 4
BH = B_PER_CORE * N_HEAD               # 32
FREE = B_PER_CORE * D_MODEL            # 2048
P = 128                                # j rows per sub-chunk (partition dim)
J_FOLD = 2                             # sub-chunks folded per DMA/iteration
SCALE = 1.0 / D_HEAD**0.5              # 0.125
CLIP = 10.0

F16 = mybir.dt.float16
F32 = mybir.dt.float32

# which HWDGE ring carries the V stream: "scalar" (ACT ring, concurrent with
# K's SP ring) or "sync" (same SP ring as K)
V_RING = "scalar"

_PROG_CACHE: dict = {}


def build_program(klen: int = KLEN):
    """Build the per-core Bass program (SPMD: same program, per-core data)."""
    rows = P * J_FOLD
    n_iter = klen // rows
    assert klen % rows == 0

    # Bacc (not plain Bass): its compile() pass splits multi-semaphore waits
    # into event-semaphore chains — TRN2 allows at most 1 wait per instruction.
    nc = bacc.Bacc()
    q_d = nc.dram_tensor("q", [1, FREE], F16, kind="ExternalInput")
    k_d = nc.dram_tensor("k", [klen, B_PER_CORE, D_MODEL], F16, kind="ExternalInput")
    v_d = nc.dram_tensor("v", [klen, B_PER_CORE, D_MODEL], F16, kind="ExternalInput")
    pv_d = nc.dram_tensor(
        "pv", [N_HEAD, B_PER_CORE, D_MODEL], F32, kind="ExternalOutput"
    )
    s_d = nc.dram_tensor("s", [BH, 1], F32, kind="ExternalOutput")

    with tile.TileContext(nc) as tc:
        with (
            tc.tile_pool(name="kv", bufs=6) as kv_pool,
            tc.tile_pool(name="small", bufs=3) as small_pool,
            tc.tile_pool(name="singles", bufs=1) as singles,
            tc.tile_pool(name="psum", bufs=1, space="PSUM") as psum_pool,
        ):
            # q replicated to all 128 partitions via broadcast DMA (SWDGE),
            # then fold-tiled on-chip (cheaper than broadcasting 2x from HBM)
            q_sb = singles.tile([P, J_FOLD, FREE], F16)
            q_ap = q_d[:]
            q_bcast = bass.AP(
                tensor=q_ap.tensor,
                offset=q_ap.offset,
                ap=[[0, P], list(q_ap.ap[-1])],
            )
            nc.gpsimd.dma_start(out=q_sb[:, 0, :], in_=q_bcast)
            for o in range(1, J_FOLD):
                nc.vector.tensor_copy(out=q_sb[:, o, :], in_=q_sb[:, 0, :])

            ones_sb = singles.tile([P, 1], F16)
            nc.vector.memset(ones_sb[:], 1.0)

            # persistent PSUM accumulators
            pv_ps = [
                psum_pool.tile([N_HEAD, D_MODEL], F32, name=f"pv{b}")
                for b in range(B_PER_CORE)
            ]
            s_ps = psum_pool.tile([BH, 1], F32, name="s")

            kv_flat = k_d[:].rearrange("j b d -> j (b d)")
            vv_flat = v_d[:].rearrange("j b d -> j (b d)")

            # fold-2 blocks for the bulk; single-P blocks at the end so the
            # serial tail compute after the last DMA is as small as possible
            blocks = []
            j0 = 0
            while klen - j0 > 2 * P:
                blocks.append((j0, J_FOLD))
                j0 += J_FOLD * P
            while j0 < klen:
                blocks.append((j0, 1))
                j0 += P

            for bi, (j0, fold) in enumerate(blocks):
                k_t = kv_pool.tile([P, fold, FREE], F16, tag="k")
                v_t = kv_pool.tile([P, fold, FREE], F16, tag="v")
                k_src = kv_flat[j0 : j0 + fold * P].rearrange(
                    "(o p) f -> p o f", p=P
                )
                v_src = vv_flat[j0 : j0 + fold * P].rearrange(
                    "(o p) f -> p o f", p=P
                )
                # K on the SP HWDGE ring, V on the ACT HWDGE ring — the two
                # physical rings run concurrently, hiding per-DMA ramp
                nc.sync.dma_start(out=k_t[:], in_=k_src)
                v_eng = nc.scalar if V_RING == "scalar" else nc.sync
                v_eng.dma_start(out=v_t[:], in_=v_src)

                # scores: k_t *= q (in place, fp16 2x mode)
                nc.vector.tensor_mul(
                    out=k_t[:], in0=k_t[:], in1=q_sb[:, 0:fold, :]
                )
                # tree-halving partial sums over d (fp16, 2x mode):
                # (p, o, g, 64) -> 32 -> 16 -> 8
                k4 = k_t[:].rearrange("p o (g d) -> p o g d", d=D_HEAD)
                nc.vector.tensor_add(
                    out=k4[:, :, :, 0:32], in0=k4[:, :, :, 0:32], in1=k4[:, :, :, 32:64]
                )
                nc.vector.tensor_add(
                    out=k4[:, :, :, 0:16], in0=k4[:, :, :, 0:16], in1=k4[:, :, :, 16:32]
                )
                nc.vector.tensor_add(
                    out=k4[:, :, :, 0:8], in0=k4[:, :, :, 0:8], in1=k4[:, :, :, 8:16]
                )
                nc.vector.tensor_add(
                    out=k4[:, :, :, 0:4], in0=k4[:, :, :, 0:4], in1=k4[:, :, :, 4:8]
                )
                # final fp32 segmented reduce of 4 -> scores (p, o*BH)
                sc = small_pool.tile([P, fold * BH], F32, tag="sc")
                nc.vector.reduce_sum(
                    out=sc[:],
                    in_=k4[:, :, :, 0:4],
                    axis=mybir.AxisListType.X,
                )
                # e = exp(CLIP * tanh(SCALE * raw_score)), fp16 for the PE
                nc.scalar.activation(
                    out=sc[:], in_=sc[:],
                    func=mybir.ActivationFunctionType.Tanh, scale=SCALE,
                )
                e = small_pool.tile([P, fold * BH], F16, tag="e")
                nc.scalar.activation(
                    out=e[:], in_=sc[:],
                    func=mybir.ActivationFunctionType.Exp, scale=CLIP,
                )

                start = bi == 0
                stop = bi == len(blocks) - 1
                for o in range(fold):
                    for b in range(B_PER_CORE):
                        nc.tensor.matmul(
                            pv_ps[b][:],
                            lhsT=e[:, o * BH + b * N_HEAD : o * BH + (b + 1) * N_HEAD],
                            rhs=v_t[:, o, b * D_MODEL : (b + 1) * D_MODEL],
                            start=start and o == 0,
                            stop=stop and o == fold - 1,
                        )
                    nc.tensor.matmul(
                        s_ps[:],
                        lhsT=e[:, o * BH : (o + 1) * BH],
                        rhs=ones_sb[:],
                        start=start and o == 0,
                        stop=stop and o == fold - 1,
                    )

            # epilogue: PSUM -> SBUF -> DRAM (fp32). The tiny s chain goes
            # first and out on the ACT ring so its DMA fixed latency overlaps
            # the pv DMA on the SP ring; pv copies split over ACT+DVE.
            s_sb = singles.tile([BH, 1], F32)
            nc.vector.tensor_copy(out=s_sb[:], in_=s_ps[:])
            nc.scalar.dma_start(out=s_d[:], in_=s_sb[:])
            pv_sb = singles.tile([N_HEAD, B_PER_CORE * D_MODEL], F32)
            for b in range(B_PER_CORE):
                eng = nc.scalar if b % 2 == 0 else nc.vector
                out_slice = pv_sb[:, b * D_MODEL : (b + 1) * D_MODEL]
                if eng is nc.scalar:
                    nc.scalar.copy(out=out_slice, in_=pv_ps[b][:])
                else:
                    nc.vector.tensor_copy(out=out_slice, in_=pv_ps[b][:])
            nc.sync.dma_start(
                out=pv_d[:].rearrange("h b d -> h (b d)"), in_=pv_sb[:]
            )
    nc.finalize()
    return nc


def shard_inputs(q: np.ndarray, k: np.ndarray, v: np.ndarray):
    """Split full inputs into per-core input maps (fp16 staging)."""
    q16 = np.asarray(q, dtype=np.float16)
    k16 = np.asarray(k, dtype=np.float16)
    v16 = np.asarray(v, dtype=np.float16)
    in_maps = []
    for i in range(N_CORES):
        b0 = i * B_PER_CORE
        in_maps.append(
            {
                "q": np.ascontiguousarray(
                    q16[0, b0 : b0 + B_PER_CORE, :]
                ).reshape(1, FREE),
                "k": np.ascontiguousarray(k16[:, b0 : b0 + B_PER_CORE, :]),
                "v": np.ascontiguousarray(v16[:, b0 : b0 + B_PER_CORE, :]),
            }
        )
    return in_maps


def combine_outputs(results) -> np.ndarray:
    """Per-core (pv, s) -> full (1, 32, 512): diagonal extract + normalize."""
    outs = []
    hh = np.arange(N_HEAD)
    for i in range(N_CORES):
        pv = np.asarray(results[i]["pv"], dtype=np.float32)
        s = np.asarray(results[i]["s"], dtype=np.float32).reshape(
            B_PER_CORE, N_HEAD
        )
        pv4 = pv.reshape(N_HEAD, B_PER_CORE, N_HEAD, D_HEAD)
        diag = pv4[hh, :, hh, :]          # (n_head, b, d_head), row h = head h
        o = diag.transpose(1, 0, 2)       # (b, h, d)
        o = o / s[:, :, None]
        outs.append(o.reshape(B_PER_CORE, D_MODEL))
    return np.concatenate(outs, axis=0)[None, :, :].astype(np.float32)


def kernel(q, k, v):
    q = np.asarray(q, dtype=np.float32)
    k = np.asarray(k, dtype=np.float32)
    v = np.asarray(v, dtype=np.float32)
    assert q.shape == (1, BSZ, D_MODEL) and k.shape == (KLEN, BSZ, D_MODEL)

    if "prog" not in _PROG_CACHE:
        _PROG_CACHE["prog"] = build_program(KLEN)
    nc = _PROG_CACHE["prog"]

    in_maps = shard_inputs(q, k, v)
    res = run_bass_kernel_spmd(nc, in_maps, list(range(N_CORES))).results
    return combine_outputs(res)


if __name__ == "__main__":
    rng = np.random.default_rng(0)
    q = rng.standard_normal((1, BSZ, D_MODEL), dtype=np.float32)
    k = rng.standard_normal((KLEN, BSZ, D_MODEL), dtype=np.float32)
    v = rng.standard_normal((KLEN, BSZ, D_MODEL), dtype=np.float32)
    out = kernel(q, k, v)
    print(out.shape, out.dtype)



# revision 11
# speedup vs baseline: 1.2540x; 1.2540x over previous
"""Trainium2 Bass kernel for single-token multi-head self-attention.

Problem (hardcoded):
  q: (1, 32, 512) f32, k/v: (8192, 32, 512) f32, 8 heads x 64 dim,
  scores = (q.k)/8, softcapped 10*tanh(.), softmax over klen, out = w.v.

Strategy: data-parallel over batch, 4 batches per core on 8 cores. The
problem is HBM-bandwidth bound, so K/V are staged to device HBM as fp8
e3m4 (quarter the fp32 traffic). K is quantized with q-weighted error
feedback along d (GPTQ-style): the running q-weighted residual is folded
into the next channel, so the *score* error telescopes to ~1e-4 even
though per-element error is ~1.3%.

Per 512-row j-block, the score work is split across engines so every
engine stays under the ~5.9us/block fp8 DMA roofline:
  - rows 0-255 (DVE half): j-major fp8 K tiles are upconverted fp8->fp16
    on ACT, then DVE computes q*k (fp16 2x), tree-halving adds over d,
    fp32 segmented reduce -> scores (128, 2*32)
  - rows 256-511 (PE half): host stages K^T in 16 chunks of 32 d-channels
    x 4 batches (partition = (b, d%32)); scores come from 16 accumulating
    PE matmuls with a block-diagonal fp16 q as the stationary operand
    (rhs = fp8 K^T straight from DMA, no upconvert) -> PSUM (32, 256),
    then 10*tanh(s/8)+exp on ACT and a PE transpose back to (128, 32) so
    e lands j-major for the PV matmul
  - P@V and sum(e) accumulate on PE into fp32 PSUM across all blocks
    (lhsT = e-slice (128,8) fp16, rhs = v-slice (128,512) fp8; ones
    column for the softmax denominator)
Epilogue ships raw PV (8, 4x512) + exp-sums (32,) to DRAM; the tiny
diagonal extraction out[b,h,:] = pv[h, b, h*64:] / s[b,h] runs on host.
"""

import numpy as np

import concourse.bass as bass
import concourse.bacc as bacc
import concourse.tile as tile
from concourse import mybir
from concourse.bass_utils import run_bass_kernel_spmd

N_CORES = 8
KLEN = 8192
BSZ = 32
D_MODEL = 512
N_HEAD = 8
D_HEAD = 64
B_PER_CORE = BSZ // N_CORES            # 4
BH = B_PER_CORE * N_HEAD               # 32
FREE = B_PER_CORE * D_MODEL            # 2048
P = 128
BLK_J = 512                            # j rows per block (4 subchunks of 128)
N_BLK = KLEN // BLK_J                  # 16
N_CHUNK = 16                           # PE half: (b, d32) contraction chunks
SCALE = 1.0 / D_HEAD**0.5              # 0.125
CLIP = 10.0

F16 = mybir.dt.float16
F32 = mybir.dt.float32
F8 = mybir.dt.float8e3
NP_F8 = mybir.dt.np(F8)  # ml_dtypes.float8_e3m4

_PROG_CACHE: dict = {}


def build_program():
    """Build the per-core Bass program (SPMD: same program, per-core data)."""
    nc = bacc.Bacc()
    q_d = nc.dram_tensor("q", [1, FREE], F16, kind="ExternalInput")
    qblk_d = nc.dram_tensor("qblk", [P, N_CHUNK, BH], F16, kind="ExternalInput")
    kj_d = nc.dram_tensor(
        "kj", [N_BLK, P, 2, FREE], F8, kind="ExternalInput"
    )
    kt_d = nc.dram_tensor(
        "kt", [N_BLK, P, N_CHUNK, 256], F8, kind="ExternalInput"
    )
    v_d = nc.dram_tensor("v", [N_BLK, P, 4, FREE], F8, kind="ExternalInput")
    ident_d = nc.dram_tensor("ident", [BH, BH], F16, kind="ExternalInput")
    pv_d = nc.dram_tensor(
        "pv", [N_HEAD, B_PER_CORE, D_MODEL], F32, kind="ExternalOutput"
    )
    s_d = nc.dram_tensor("s", [BH, 1], F32, kind="ExternalOutput")

    with tile.TileContext(nc) as tc:
        with (
            tc.tile_pool(name="kv", bufs=6) as kv_pool,
            tc.tile_pool(name="small", bufs=4) as small_pool,
            tc.tile_pool(name="singles", bufs=1) as singles,
            tc.tile_pool(name="psum", bufs=1, space="PSUM") as psum_pool,
        ):
            # q replicated to all 128 partitions via broadcast DMA (SWDGE),
            # then fold-tiled on-chip
            q_sb = singles.tile([P, 2, FREE], F16)
            q_ap = q_d[:]
            q_bcast = bass.AP(
                tensor=q_ap.tensor,
                offset=q_ap.offset,
                ap=[[0, P], list(q_ap.ap[-1])],
            )
            nc.gpsimd.dma_start(out=q_sb[:, 0, :], in_=q_bcast)
            nc.vector.tensor_copy(out=q_sb[:, 1, :], in_=q_sb[:, 0, :])

            qblk_sb = singles.tile([P, N_CHUNK, BH], F16)
            nc.sync.dma_start(out=qblk_sb[:], in_=qblk_d[:])

            ones_sb = singles.tile([P, 1], F16)
            nc.vector.memset(ones_sb[:], 1.0)
            ident_sb = singles.tile([BH, BH], F16)
            nc.sync.dma_start(out=ident_sb[:], in_=ident_d[:])

            # persistent PSUM accumulators
            pv_ps = [
                psum_pool.tile([N_HEAD, D_MODEL], F32, name=f"pv{b}")
                for b in range(B_PER_CORE)
            ]
            s_ps = psum_pool.tile([BH, 1], F32, name="s")
            sc_pe_ps = [
                psum_pool.tile([BH, 256], F32, name=f"scpe{i}") for i in range(2)
            ]
            eT_ps = psum_pool.tile([P, 2, BH], F16, name="eT")

            for blk in range(N_BLK):
                kj8 = kv_pool.tile([P, 2, FREE], F8, tag="kj8")
                kt8 = kv_pool.tile([P, N_CHUNK, 256], F8, tag="kt8")
                v8 = kv_pool.tile([P, 4, FREE], F8, tag="v8")
                k16 = kv_pool.tile([P, 2, FREE], F16, tag="k16")

                nc.sync.dma_start(out=kj8[:], in_=kj_d[blk])
                nc.sync.dma_start(out=kt8[:], in_=kt_d[blk])
                nc.scalar.dma_start(out=v8[:], in_=v_d[blk])

                # ---- DVE half: subchunks 0,1 (j-major) ----
                nc.scalar.copy(out=k16[:, 0, :], in_=kj8[:, 0, :])
                nc.scalar.copy(out=k16[:, 1, :], in_=kj8[:, 1, :])
                nc.vector.tensor_mul(out=k16[:], in0=k16[:], in1=q_sb[:])
                k4 = k16[:].rearrange("p o (g d) -> p o g d", d=D_HEAD)
                nc.vector.tensor_add(
                    out=k4[:, :, :, 0:32], in0=k4[:, :, :, 0:32], in1=k4[:, :, :, 32:64]
                )
                nc.vector.tensor_add(
                    out=k4[:, :, :, 0:16], in0=k4[:, :, :, 0:16], in1=k4[:, :, :, 16:32]
                )
                nc.vector.tensor_add(
                    out=k4[:, :, :, 0:8], in0=k4[:, :, :, 0:8], in1=k4[:, :, :, 8:16]
                )
                nc.vector.tensor_add(
                    out=k4[:, :, :, 0:4], in0=k4[:, :, :, 0:4], in1=k4[:, :, :, 4:8]
                )
                sc = small_pool.tile([P, 2 * BH], F32, tag="sc")
                nc.vector.reduce_sum(
                    out=sc[:], in_=k4[:, :, :, 0:4], axis=mybir.AxisListType.X
                )
                nc.scalar.activation(
                    out=sc[:], in_=sc[:],
                    func=mybir.ActivationFunctionType.Tanh, scale=SCALE,
                )
                e_dve = small_pool.tile([P, 2 * BH], F16, tag="e")
                nc.scalar.activation(
                    out=e_dve[:], in_=sc[:],
                    func=mybir.ActivationFunctionType.Exp, scale=CLIP,
                )

                # ---- PE half: subchunks 2,3 (kT-mixed chunks) ----
                scp = sc_pe_ps[blk % 2]
                for c in range(N_CHUNK):
                    nc.tensor.matmul(
                        scp[:],
                        lhsT=qblk_sb[:, c, :],
                        rhs=kt8[:, c, :],
                        start=c == 0,
                        stop=c == N_CHUNK - 1,
                    )
                scs = small_pool.tile([BH, 256], F32, tag="scpe_sb")
                nc.scalar.activation(
                    out=scs[:], in_=scp[:],
                    func=mybir.ActivationFunctionType.Tanh, scale=SCALE,
                )
                e_pe = small_pool.tile([BH, 256], F16, tag="epe")
                nc.scalar.activation(
                    out=e_pe[:], in_=scs[:],
                    func=mybir.ActivationFunctionType.Exp, scale=CLIP,
                )
                # transpose e_pe (32, 2*128) -> eT (128, 2*32), j-major
                for t in range(2):
                    nc.tensor.transpose(
                        eT_ps[:, t, :],
                        e_pe[:, t * P : (t + 1) * P],
                        ident_sb[:],
                    )
                eT = small_pool.tile([P, 2, BH], F16, tag="eT_sb")
                nc.vector.tensor_copy(out=eT[:], in_=eT_ps[:])

                # ---- PV + denominator on PE (all 4 subchunks) ----
                start = blk == 0
                stop = blk == N_BLK - 1
                for o in range(4):
                    if o < 2:
                        e_sl = e_dve[:, o * BH : (o + 1) * BH]
                    else:
                        e_sl = eT[:, o - 2, :]
                    for b in range(B_PER_CORE):
                        nc.tensor.matmul(
                            pv_ps[b][:],
                            lhsT=e_sl[:, b * N_HEAD : (b + 1) * N_HEAD],
                            rhs=v8[:, o, b * D_MODEL : (b + 1) * D_MODEL],
                            start=start and o == 0,
                            stop=stop and o == 3,
                        )
                    nc.tensor.matmul(
                        s_ps[:],
                        lhsT=e_sl[:],
                        rhs=ones_sb[:],
                        start=start and o == 0,
                        stop=stop and o == 3,
                    )

            # epilogue: PSUM -> SBUF -> DRAM (fp32).
            s_sb = singles.tile([BH, 1], F32)
            nc.vector.tensor_copy(out=s_sb[:], in_=s_ps[:])
            nc.scalar.dma_start(out=s_d[:], in_=s_sb[:])
            pv_sb = singles.tile([N_HEAD, B_PER_CORE * D_MODEL], F32)
            for b in range(B_PER_CORE):
                out_slice = pv_sb[:, b * D_MODEL : (b + 1) * D_MODEL]
                if b % 2 == 0:
                    nc.scalar.copy(out=out_slice, in_=pv_ps[b][:])
                else:
                    nc.vector.tensor_copy(out=out_slice, in_=pv_ps[b][:])
            nc.sync.dma_start(
                out=pv_d[:].rearrange("h b d -> h (b d)"), in_=pv_sb[:]
            )
    nc.finalize()
    return nc


def feedback_quant_k(k: np.ndarray, q16: np.ndarray) -> np.ndarray:
    """e3m4-quantize K with q-weighted error feedback along d.

    For each (j, b, h) the running residual c = sum_d q_d (k̂_d - k_d) is
    subtracted from the next channel before rounding (descending-|q| channel
    order), so the score error telescopes to the last channel's rounding.
    Returns the e3m4 array (J, 32, 512).
    """
    J = k.shape[0]
    kh = k.reshape(J, BSZ, N_HEAD, D_HEAD).astype(np.float32)
    qh = q16.reshape(BSZ, N_HEAD, D_HEAD).astype(np.float32)
    out = np.empty((J, BSZ, N_HEAD, D_HEAD), dtype=NP_F8)
    order = np.argsort(-np.abs(qh), axis=-1)          # (32, 8, 64)
    c = np.zeros((J, BSZ, N_HEAD), dtype=np.float32)
    b_idx = np.arange(BSZ)[:, None]
    h_idx = np.arange(N_HEAD)[None, :]
    for t in range(D_HEAD):
        d = order[:, :, t]                            # (32, 8)
        qt = qh[b_idx, h_idx, d]                      # (32, 8)
        kt = kh[:, b_idx, h_idx, d]                   # (J, 32, 8)
        safe = np.abs(qt) > 1e-3
        adj = np.where(safe, c / np.where(safe, qt, 1.0), 0.0)
        adj = np.clip(adj, -0.25, 0.25)
        khat8 = (kt - adj).astype(NP_F8)
        c = c + qt * (khat8.astype(np.float32) - kt)
        out[:, b_idx, h_idx, d] = khat8
    return out.reshape(J, BSZ, D_MODEL)


def shard_inputs(q: np.ndarray, k: np.ndarray, v: np.ndarray):
    """Stage full inputs into per-core maps (q fp16, K/V fp8 e3m4)."""
    q16 = np.asarray(q, dtype=np.float16)
    k8 = feedback_quant_k(np.asarray(k, dtype=np.float32), q16[0])
    v8 = np.asarray(v, dtype=NP_F8)

    in_maps = []
    for i in range(N_CORES):
        b0 = i * B_PER_CORE
        qc = q16[0, b0 : b0 + B_PER_CORE, :]              # (4, 512)
        kc = k8[:, b0 : b0 + B_PER_CORE, :]               # (8192, 4, 512)
        vc = v8[:, b0 : b0 + B_PER_CORE, :]

        # blocks of 512 j rows: subchunks (o) of 128: j = blk*512 + o*128 + p
        kb = np.ascontiguousarray(kc).reshape(N_BLK, 4, P, 4, D_MODEL)
        kb = kb.transpose(0, 2, 1, 3, 4)                  # (blk, p, o, b, d)
        kj = np.ascontiguousarray(
            kb[:, :, 0:2].reshape(N_BLK, P, 2, FREE)
        )
        # PE half: subchunks 2,3 -> (blk, p=(b*32+dsub), chunk=(d//32), jj)
        # jj in [0,256): jj = (o-2)*128 + p_row
        kpe = kb[:, :, 2:4]                               # (blk, p, 2, b, d)
        kpe = kpe.reshape(N_BLK, P, 2, 4, N_CHUNK, 32)    # d -> (chunk, dsub)
        # target [blk][pp=(b,dsub)][chunk][jj=(o,p)]
        kt = kpe.transpose(0, 3, 5, 4, 2, 1)              # blk,b,dsub,chunk,o,p
        kt = np.ascontiguousarray(kt.reshape(N_BLK, P, N_CHUNK, 256))

        vb = np.ascontiguousarray(vc).reshape(N_BLK, 4, P, 4, D_MODEL)
        vb = np.ascontiguousarray(
            vb.transpose(0, 2, 1, 3, 4).reshape(N_BLK, P, 4, FREE)
        )

        # block-diagonal q for the PE half: qblk[p=(b,dsub), c, col=(b'*8+h')]
        qblk = np.zeros((P, N_CHUNK, BH), dtype=np.float16)
        for c in range(N_CHUNK):
            h = c // 2
            for b in range(4):
                for dsub in range(32):
                    qblk[b * 32 + dsub, c, b * N_HEAD + h] = qc[
                        b, c * 32 + dsub
                    ]

        in_maps.append(
            {
                "q": np.ascontiguousarray(qc).reshape(1, FREE),
                "qblk": qblk,
                "kj": kj,
                "kt": kt,
                "v": vb,
                "ident": np.eye(BH, dtype=np.float16),
            }
        )
    return in_maps


def combine_outputs(results) -> np.ndarray:
    """Per-core (pv, s) -> full (1, 32, 512): diagonal extract + normalize."""
    outs = []
    hh = np.arange(N_HEAD)
    for i in range(N_CORES):
        pv = np.asarray(results[i]["pv"], dtype=np.float32)
        s = np.asarray(results[i]["s"], dtype=np.float32).reshape(
            B_PER_CORE, N_HEAD
        )
        pv4 = pv.reshape(N_HEAD, B_PER_CORE, N_HEAD, D_HEAD)
        diag = pv4[hh, :, hh, :]          # (n_head, b, d_head), row h = head h
        o = diag.transpose(1, 0, 2)       # (b, h, d)
        o = o / s[:, :, None]
        outs.append(o.reshape(B_PER_CORE, D_MODEL))
    return np.concatenate(outs, axis=0)[None, :, :].astype(np.float32)


def kernel(q, k, v):
    q = np.asarray(q, dtype=np.float32)
    k = np.asarray(k, dtype=np.float32)
    v = np.asarray(v, dtype=np.float32)
    assert q.shape == (1, BSZ, D_MODEL) and k.shape == (KLEN, BSZ, D_MODEL)

    if "prog" not in _PROG_CACHE:
        _PROG_CACHE["prog"] = build_program()
    nc = _PROG_CACHE["prog"]

    in_maps = shard_inputs(q, k, v)
    res = run_bass_kernel_spmd(nc, in_maps, list(range(N_CORES))).results
    return combine_outputs(res)


if __name__ == "__main__":
    rng = np.random.default_rng(0)
    q = rng.standard_normal((1, BSZ, D_MODEL), dtype=np.float32)
    k = rng.standard_normal((KLEN, BSZ, D_MODEL), dtype=np.float32)
    v = rng.standard_normal((KLEN, BSZ, D_MODEL), dtype=np.float32)
    out = kernel(q, k, v)
    print(out.shape, out.dtype)


# revision 22
# speedup vs baseline: 1.4026x; 1.1185x over previous
"""Trainium2 Bass kernel for single-token multi-head self-attention.

Problem (hardcoded):
  q: (1, 32, 512) f32, k/v: (8192, 32, 512) f32, 8 heads x 64 dim,
  scores = (q.k)/8, softcapped 10*tanh(.), softmax over klen, out = w.v.

Strategy: data-parallel over batch, 4 batches per core on 8 cores. The
problem is HBM-bandwidth bound, so K/V are staged to device HBM as fp8
e3m4 (quarter the fp32 traffic). K is quantized with q-weighted error
feedback along d (GPTQ-style): the running q-weighted residual is folded
into the next channel, so the *score* error telescopes to ~1e-4 even
though per-element error is ~1.3%.

Per 512-row j-block, the score work is split across engines so every
engine stays under the ~5.9us/block fp8 DMA roofline:
  - rows 0-255 (DVE half): j-major fp8 K tiles are upconverted fp8->fp16
    on ACT, then DVE computes q*k (fp16 2x), tree-halving adds over d,
    fp32 segmented reduce -> scores (128, 2*32)
  - rows 256-511 (PE half): host stages K^T in 16 chunks of 32 d-channels
    x 4 batches (partition = (b, d%32)); scores come from 16 accumulating
    PE matmuls with a block-diagonal fp16 q as the stationary operand
    (rhs = fp8 K^T straight from DMA, no upconvert) -> PSUM (32, 256),
    then 10*tanh(s/8)+exp on ACT and a PE transpose back to (128, 32) so
    e lands j-major for the PV matmul
  - P@V and sum(e) accumulate on PE into fp32 PSUM across all blocks
    (lhsT = e-slice (128,8) fp16, rhs = v-slice (128,512) fp8; ones
    column for the softmax denominator)
Epilogue ships raw PV (8, 4x512) + exp-sums (32,) to DRAM; the tiny
diagonal extraction out[b,h,:] = pv[h, b, h*64:] / s[b,h] runs on host.
"""

import numpy as np

import concourse.bass as bass
import concourse.bacc as bacc
import concourse.tile as tile
from concourse import mybir
from concourse.bass_utils import run_bass_kernel_spmd

N_CORES = 8
KLEN = 8192
BSZ = 32
D_MODEL = 512
N_HEAD = 8
D_HEAD = 64
B_PER_CORE = BSZ // N_CORES            # 4
BH = B_PER_CORE * N_HEAD               # 32
FREE = B_PER_CORE * D_MODEL            # 2048
P = 128
BLK_J = 512                            # j rows per block (4 subchunks of 128)
N_BLK = KLEN // BLK_J                  # 16
N_CHUNK = 16                           # PE half: (b, d32) contraction chunks
SCALE = 1.0 / D_HEAD**0.5              # 0.125
CLIP = 10.0

F16 = mybir.dt.float16
F32 = mybir.dt.float32
F8 = mybir.dt.float8e3
NP_F8 = mybir.dt.np(F8)  # ml_dtypes.float8_e3m4

_PROG_CACHE: dict = {}


def build_program():
    """Build the per-core Bass program (SPMD: same program, per-core data)."""
    nc = bacc.Bacc()
    q_d = nc.dram_tensor("q", [1, FREE], F16, kind="ExternalInput")
    qblk_d = nc.dram_tensor("qblk", [P, N_CHUNK, BH], F16, kind="ExternalInput")
    kj_d = nc.dram_tensor(
        "kj", [N_BLK, P, 2, FREE], F8, kind="ExternalInput"
    )
    kt_d = nc.dram_tensor(
        "kt", [N_BLK, P, N_CHUNK, 256], F8, kind="ExternalInput"
    )
    v_d = nc.dram_tensor("v", [N_BLK, P, 4, FREE], F8, kind="ExternalInput")
    ident_d = nc.dram_tensor("ident", [BH, BH], F16, kind="ExternalInput")
    pv_d = nc.dram_tensor("pv", [P, FREE], F32, kind="ExternalOutput")
    s_d = nc.dram_tensor("s", [1, 4 * BH], F32, kind="ExternalOutput")

    with tile.TileContext(nc) as tc:
        with (
            tc.tile_pool(name="kv", bufs=6) as kv_pool,
            tc.tile_pool(name="small", bufs=4) as small_pool,
            tc.tile_pool(name="singles", bufs=1) as singles,
            tc.tile_pool(name="psum", bufs=1, space="PSUM") as psum_pool,
        ):
            # q replicated to all 128 partitions via broadcast DMA (SWDGE),
            # then fold-tiled on-chip
            q_sb = singles.tile([P, 2, FREE], F16)
            q_ap = q_d[:]
            q_bcast = bass.AP(
                tensor=q_ap.tensor,
                offset=q_ap.offset,
                ap=[[0, P], list(q_ap.ap[-1])],
            )
            nc.gpsimd.dma_start(out=q_sb[:, 0, :], in_=q_bcast)
            nc.vector.tensor_copy(out=q_sb[:, 1, :], in_=q_sb[:, 0, :])

            qblk_sb = singles.tile([P, N_CHUNK, BH], F16)
            nc.sync.dma_start(out=qblk_sb[:], in_=qblk_d[:])

            ones_sb = singles.tile([P, 1], F16)
            nc.vector.memset(ones_sb[:], 1.0)
            ident_sb = singles.tile([BH, BH], F16)
            nc.sync.dma_start(out=ident_sb[:], in_=ident_d[:])

            # persistent PSUM accumulators. pv rows are (o, b, h): the four
            # subchunk lanes run as concurrent PE column-group tiles and the
            # host sums the four o-groups (and extracts the b==b' diagonal)
            pv_ps = psum_pool.tile([P, FREE], F32, name="pv")
            s_ps = psum_pool.tile([1, 4 * BH], F32, name="s")
            sc_pe_ps = [
                psum_pool.tile([BH, 256], F32, name=f"scpe{i}") for i in range(2)
            ]
            eT_ps = psum_pool.tile([P, 2, BH], F16, name="eT")

            for blk in range(N_BLK):
                kj8 = kv_pool.tile([P, 2, FREE], F8, tag="kj8")
                kt8 = kv_pool.tile([P, N_CHUNK, 256], F8, tag="kt8")
                v8 = kv_pool.tile([P, 4, FREE], F8, tag="v8")
                k16 = kv_pool.tile([P, 2, FREE], F16, tag="k16")

                nc.sync.dma_start(out=kj8[:], in_=kj_d[blk])
                nc.sync.dma_start(out=kt8[:], in_=kt_d[blk])
                nc.scalar.dma_start(out=v8[:], in_=v_d[blk])

                # ---- DVE half: subchunks 0,1 (j-major) ----
                nc.scalar.copy(out=k16[:, 0, :], in_=kj8[:, 0, :])
                nc.scalar.copy(out=k16[:, 1, :], in_=kj8[:, 1, :])
                nc.vector.tensor_mul(out=k16[:], in0=k16[:], in1=q_sb[:])
                k4 = k16[:].rearrange("p o (g d) -> p o g d", d=D_HEAD)
                nc.vector.tensor_add(
                    out=k4[:, :, :, 0:32], in0=k4[:, :, :, 0:32], in1=k4[:, :, :, 32:64]
                )
                nc.vector.tensor_add(
                    out=k4[:, :, :, 0:16], in0=k4[:, :, :, 0:16], in1=k4[:, :, :, 16:32]
                )
                nc.vector.tensor_add(
                    out=k4[:, :, :, 0:8], in0=k4[:, :, :, 0:8], in1=k4[:, :, :, 8:16]
                )
                nc.vector.tensor_add(
                    out=k4[:, :, :, 0:4], in0=k4[:, :, :, 0:4], in1=k4[:, :, :, 4:8]
                )
                sc = small_pool.tile([P, 2 * BH], F32, tag="sc")
                nc.vector.reduce_sum(
                    out=sc[:], in_=k4[:, :, :, 0:4], axis=mybir.AxisListType.X
                )
                nc.scalar.activation(
                    out=sc[:], in_=sc[:],
                    func=mybir.ActivationFunctionType.Tanh, scale=SCALE,
                )
                # e for all 4 subchunks lands j-major in one tile so the
                # denominator is a single PE matmul per block
                e_blk = small_pool.tile([P, 4, BH], F16, tag="e")
                nc.scalar.activation(
                    out=e_blk[:, 0:2, :], in_=sc[:],
                    func=mybir.ActivationFunctionType.Exp, scale=CLIP,
                )

                # ---- PE half: subchunks 2,3 (kT-mixed chunks) ----
                scp = sc_pe_ps[blk % 2]
                for c in range(N_CHUNK):
                    nc.tensor.matmul(
                        scp[:],
                        lhsT=qblk_sb[:, c, :],
                        rhs=kt8[:, c, :],
                        start=c == 0,
                        stop=c == N_CHUNK - 1,
                    )
                scs = small_pool.tile([BH, 256], F32, tag="scpe_sb")
                nc.scalar.activation(
                    out=scs[:], in_=scp[:],
                    func=mybir.ActivationFunctionType.Tanh, scale=SCALE,
                )
                e_pe = small_pool.tile([BH, 256], F16, tag="epe")
                nc.scalar.activation(
                    out=e_pe[:], in_=scs[:],
                    func=mybir.ActivationFunctionType.Exp, scale=CLIP,
                )
                # transpose e_pe (32, 2*128) -> eT (128, 2*32), j-major
                for t in range(2):
                    nc.tensor.transpose(
                        eT_ps[:, t, :],
                        e_pe[:, t * P : (t + 1) * P],
                        ident_sb[:],
                    )
                nc.vector.tensor_copy(out=e_blk[:, 2:4, :], in_=eT_ps[:])

                # ---- PV + denominator on PE (all 4 subchunks) ----
                # 4 o-lanes on distinct 32-wide PE column groups, concurrent
                start = blk == 0
                stop = blk == N_BLK - 1
                for ds in range(4):
                    dsl = slice(ds * D_MODEL, (ds + 1) * D_MODEL)
                    for o in range(4):
                        nc.tensor.matmul(
                            pv_ps[o * BH : (o + 1) * BH, dsl],
                            lhsT=e_blk[:, o, :],
                            rhs=v8[:, o, dsl],
                            start=start,
                            stop=stop,
                            tile_position=(0, o * BH),
                        )
                nc.tensor.matmul(
                    s_ps[:],
                    lhsT=ones_sb[:],
                    rhs=e_blk[:].rearrange("p o c -> p (o c)"),
                    start=start,
                    stop=stop,
                )

            # epilogue: PSUM -> SBUF -> DRAM (fp32).
            s_sb = singles.tile([1, 4 * BH], F32)
            nc.vector.tensor_copy(out=s_sb[:], in_=s_ps[:])
            nc.scalar.dma_start(out=s_d[:], in_=s_sb[:])
            pv_sb = singles.tile([P, FREE], F32)
            nc.scalar.copy(out=pv_sb[:, 0:1024], in_=pv_ps[:, 0:1024])
            nc.vector.tensor_copy(out=pv_sb[:, 1024:2048], in_=pv_ps[:, 1024:2048])
            nc.sync.dma_start(out=pv_d[:], in_=pv_sb[:])
    nc.finalize()
    return nc


def feedback_quant_k(k: np.ndarray, q16: np.ndarray) -> np.ndarray:
    """e3m4-quantize K with q-weighted error feedback along d.

    For each (j, b, h) the running residual c = sum_d q_d (k̂_d - k_d) is
    subtracted from the next channel before rounding (descending-|q| channel
    order), so the score error telescopes to the last channel's rounding.
    Returns the e3m4 array (J, 32, 512).
    """
    J = k.shape[0]
    kh = k.reshape(J, BSZ, N_HEAD, D_HEAD).astype(np.float32)
    qh = q16.reshape(BSZ, N_HEAD, D_HEAD).astype(np.float32)
    out = np.empty((J, BSZ, N_HEAD, D_HEAD), dtype=NP_F8)
    order = np.argsort(-np.abs(qh), axis=-1)          # (32, 8, 64)
    c = np.zeros((J, BSZ, N_HEAD), dtype=np.float32)
    b_idx = np.arange(BSZ)[:, None]
    h_idx = np.arange(N_HEAD)[None, :]
    for t in range(D_HEAD):
        d = order[:, :, t]                            # (32, 8)
        qt = qh[b_idx, h_idx, d]                      # (32, 8)
        kt = kh[:, b_idx, h_idx, d]                   # (J, 32, 8)
        safe = np.abs(qt) > 1e-3
        adj = np.where(safe, c / np.where(safe, qt, 1.0), 0.0)
        adj = np.clip(adj, -0.25, 0.25)
        khat8 = (kt - adj).astype(NP_F8)
        c = c + qt * (khat8.astype(np.float32) - kt)
        out[:, b_idx, h_idx, d] = khat8
    return out.reshape(J, BSZ, D_MODEL)


def shard_inputs(q: np.ndarray, k: np.ndarray, v: np.ndarray):
    """Stage full inputs into per-core maps (q fp16, K/V fp8 e3m4)."""
    q16 = np.asarray(q, dtype=np.float16)
    k8 = feedback_quant_k(np.asarray(k, dtype=np.float32), q16[0])
    v8 = np.asarray(v, dtype=NP_F8)

    in_maps = []
    for i in range(N_CORES):
        b0 = i * B_PER_CORE
        qc = q16[0, b0 : b0 + B_PER_CORE, :]              # (4, 512)
        kc = k8[:, b0 : b0 + B_PER_CORE, :]               # (8192, 4, 512)
        vc = v8[:, b0 : b0 + B_PER_CORE, :]

        # blocks of 512 j rows: subchunks (o) of 128: j = blk*512 + o*128 + p
        kb = np.ascontiguousarray(kc).reshape(N_BLK, 4, P, 4, D_MODEL)
        kb = kb.transpose(0, 2, 1, 3, 4)                  # (blk, p, o, b, d)
        kj = np.ascontiguousarray(
            kb[:, :, 0:2].reshape(N_BLK, P, 2, FREE)
        )
        # PE half: subchunks 2,3 -> (blk, p=(b*32+dsub), chunk=(d//32), jj)
        # jj in [0,256): jj = (o-2)*128 + p_row
        kpe = kb[:, :, 2:4]                               # (blk, p, 2, b, d)
        kpe = kpe.reshape(N_BLK, P, 2, 4, N_CHUNK, 32)    # d -> (chunk, dsub)
        # target [blk][pp=(b,dsub)][chunk][jj=(o,p)]
        kt = kpe.transpose(0, 3, 5, 4, 2, 1)              # blk,b,dsub,chunk,o,p
        kt = np.ascontiguousarray(kt.reshape(N_BLK, P, N_CHUNK, 256))

        vb = np.ascontiguousarray(vc).reshape(N_BLK, 4, P, 4, D_MODEL)
        vb = np.ascontiguousarray(
            vb.transpose(0, 2, 1, 3, 4).reshape(N_BLK, P, 4, FREE)
        )

        # block-diagonal q for the PE half: qblk[p=(b,dsub), c, col=(b'*8+h')]
        qblk = np.zeros((P, N_CHUNK, BH), dtype=np.float16)
        for c in range(N_CHUNK):
            h = c // 2
            for b in range(4):
                for dsub in range(32):
                    qblk[b * 32 + dsub, c, b * N_HEAD + h] = qc[
                        b, c * 32 + dsub
                    ]

        in_maps.append(
            {
                "q": np.ascontiguousarray(qc).reshape(1, FREE),
                "qblk": qblk,
                "kj": kj,
                "kt": kt,
                "v": vb,
                "ident": np.eye(BH, dtype=np.float16),
            }
        )
    return in_maps


def combine_outputs(results) -> np.ndarray:
    """Per-core (pv, s) -> full (1, 32, 512): diagonal extract + normalize."""
    outs = []
    hh = np.arange(N_HEAD)
    for i in range(N_CORES):
        pv = (
            np.asarray(results[i]["pv"], dtype=np.float32)
            .reshape(4, BH, FREE)
            .sum(axis=0)
        )
        s = (
            np.asarray(results[i]["s"], dtype=np.float32)
            .reshape(4, B_PER_CORE, N_HEAD)
            .sum(axis=0)
        )
        # pv[(b,h), (b',d)] -> keep b'==b: (b, h, d_model) then head slice
        pv4 = pv.reshape(B_PER_CORE, N_HEAD, B_PER_CORE, N_HEAD, D_HEAD)
        bb = np.arange(B_PER_CORE)
        diag = pv4[bb, :, bb]             # (b, h, n_head, d_head)
        o = diag[:, hh, hh, :]            # (b, h, d_head) per-head slice
        o = o / s[:, :, None]
        outs.append(o.reshape(B_PER_CORE, D_MODEL))
    return np.concatenate(outs, axis=0)[None, :, :].astype(np.float32)


def kernel(q, k, v):
    q = np.asarray(q, dtype=np.float32)
    k = np.asarray(k, dtype=np.float32)
    v = np.asarray(v, dtype=np.float32)
    assert q.shape == (1, BSZ, D_MODEL) and k.shape == (KLEN, BSZ, D_MODEL)

    if "prog" not in _PROG_CACHE:
        _PROG_CACHE["prog"] = build_program()
    nc = _PROG_CACHE["prog"]

    in_maps = shard_inputs(q, k, v)
    res = run_bass_kernel_spmd(nc, in_maps, list(range(N_CORES))).results
    return combine_outputs(res)


if __name__ == "__main__":
    rng = np.random.default_rng(0)
    q = rng.standard_normal((1, BSZ, D_MODEL), dtype=np.float32)
    k = rng.standard_normal((KLEN, BSZ, D_MODEL), dtype=np.float32)
    v = rng.standard_normal((KLEN, BSZ, D_MODEL), dtype=np.float32)
    out = kernel(q, k, v)
    print(out.shape, out.dtype)


# revision 24
# speedup vs baseline: 1.6421x; 1.1708x over previous
"""Trainium2 Bass kernel for single-token multi-head self-attention.

Problem (hardcoded):
  q: (1, 32, 512) f32, k/v: (8192, 32, 512) f32, 8 heads x 64 dim,
  scores = (q.k)/8, softcapped 10*tanh(.), softmax over klen, out = w.v.

Strategy: data-parallel over batch, 4 batches per core on 8 cores. The
problem is HBM-bandwidth bound, so K/V are staged to device HBM as fp8
e3m4 (quarter the fp32 traffic). K is quantized with q-weighted error
feedback along d (GPTQ-style): the running q-weighted residual is folded
into the next channel, so the *score* error telescopes to ~1e-4 even
though per-element error is ~1.3%.

Per 512-row j-block, the score work is split across engines so every
engine stays under the ~5.9us/block fp8 DMA roofline:
  - rows 0-255 (DVE half): j-major fp8 K tiles are upconverted fp8->fp16
    on ACT, then DVE computes q*k (fp16 2x), tree-halving adds over d,
    fp32 segmented reduce -> scores (128, 2*32)
  - rows 256-511 (PE half): host stages K^T in 16 chunks of 32 d-channels
    x 4 batches (partition = (b, d%32)); scores come from 16 accumulating
    PE matmuls with a block-diagonal fp16 q as the stationary operand
    (rhs = fp8 K^T straight from DMA, no upconvert) -> PSUM (32, 256),
    then 10*tanh(s/8)+exp on ACT and a PE transpose back to (128, 32) so
    e lands j-major for the PV matmul
  - P@V and sum(e) accumulate on PE into fp32 PSUM across all blocks
    (lhsT = e-slice (128,8) fp16, rhs = v-slice (128,512) fp8; ones
    column for the softmax denominator)
Epilogue ships raw PV (8, 4x512) + exp-sums (32,) to DRAM; the tiny
diagonal extraction out[b,h,:] = pv[h, b, h*64:] / s[b,h] runs on host.
"""

import numpy as np

import concourse.bass as bass
import concourse.bacc as bacc
import concourse.tile as tile
from concourse import mybir
from concourse.bass_utils import run_bass_kernel_spmd

N_CORES = 8
KLEN = 8192
BSZ = 32
D_MODEL = 512
N_HEAD = 8
D_HEAD = 64
B_PER_CORE = BSZ // N_CORES            # 4
BH = B_PER_CORE * N_HEAD               # 32
FREE = B_PER_CORE * D_MODEL            # 2048
P = 128
BLK_J = 512                            # j rows per block (4 subchunks of 128)
N_BLK = KLEN // BLK_J                  # 16
N_CHUNK = 16                           # PE half: (b, d32) contraction chunks
SCALE = 1.0 / D_HEAD**0.5              # 0.125
CLIP = 10.0

F16 = mybir.dt.float16
F32 = mybir.dt.float32
F8 = mybir.dt.float8e3
NP_F8 = mybir.dt.np(F8)  # ml_dtypes.float8_e3m4

_PROG_CACHE: dict = {}


def build_program():
    """Build the per-core Bass program (SPMD: same program, per-core data)."""
    nc = bacc.Bacc()
    q_d = nc.dram_tensor("q", [1, FREE], F16, kind="ExternalInput")
    qblk_d = nc.dram_tensor("qblk", [P, N_CHUNK, BH], F16, kind="ExternalInput")
    kj_d = nc.dram_tensor("kj", [N_BLK, P, FREE], F8, kind="ExternalInput")
    kt_d = nc.dram_tensor(
        "kt", [N_BLK, P, N_CHUNK, 3 * P], F8, kind="ExternalInput"
    )
    v_d = nc.dram_tensor("v", [N_BLK, P, 4, FREE], F8, kind="ExternalInput")
    ident_d = nc.dram_tensor("ident", [BH, BH], F16, kind="ExternalInput")
    pv_d = nc.dram_tensor("pv", [P, FREE], F32, kind="ExternalOutput")
    s_d = nc.dram_tensor("s", [1, 4 * BH], F32, kind="ExternalOutput")

    with tile.TileContext(nc) as tc:
        with (
            tc.tile_pool(name="kv", bufs=6) as kv_pool,
            tc.tile_pool(name="small", bufs=4) as small_pool,
            tc.tile_pool(name="singles", bufs=1) as singles,
            tc.tile_pool(name="psum", bufs=1, space="PSUM") as psum_pool,
        ):
            # q replicated to all 128 partitions via broadcast DMA (SWDGE),
            # then fold-tiled on-chip
            q_sb = singles.tile([P, FREE], F16)
            q_ap = q_d[:]
            q_bcast = bass.AP(
                tensor=q_ap.tensor,
                offset=q_ap.offset,
                ap=[[0, P], list(q_ap.ap[-1])],
            )
            nc.gpsimd.dma_start(out=q_sb[:], in_=q_bcast)

            qblk_sb = singles.tile([P, N_CHUNK, BH], F16)
            nc.sync.dma_start(out=qblk_sb[:], in_=qblk_d[:])

            ones_sb = singles.tile([P, 1], F16)
            nc.vector.memset(ones_sb[:], 1.0)
            ident_sb = singles.tile([BH, BH], F16)
            nc.sync.dma_start(out=ident_sb[:], in_=ident_d[:])

            # persistent PSUM accumulators. pv rows are (o, b, h): the four
            # subchunk lanes run as concurrent PE column-group tiles and the
            # host sums the four o-groups (and extracts the b==b' diagonal)
            pv_ps = psum_pool.tile([P, FREE], F32, name="pv")
            s_ps = psum_pool.tile([1, 4 * BH], F32, name="s")
            sc_pe_ps = [
                psum_pool.tile([BH, 3 * P], F32, name=f"scpe{i}") for i in range(2)
            ]
            eT_ps = psum_pool.tile([P, 3, BH], F16, name="eT")

            for blk in range(N_BLK):
                kj8 = kv_pool.tile([P, FREE], F8, tag="kj8")
                kt8 = kv_pool.tile([P, N_CHUNK, 3 * P], F8, tag="kt8")
                v8 = kv_pool.tile([P, 4, FREE], F8, tag="v8")
                k16 = kv_pool.tile([P, FREE], F16, tag="k16")

                nc.sync.dma_start(out=kj8[:], in_=kj_d[blk])
                nc.sync.dma_start(out=kt8[:], in_=kt_d[blk])
                nc.scalar.dma_start(out=v8[:], in_=v_d[blk])

                # ---- DVE half: subchunk 0 (j-major) ----
                nc.scalar.copy(out=k16[:], in_=kj8[:])
                nc.vector.tensor_mul(out=k16[:], in0=k16[:], in1=q_sb[:])
                k4 = k16[:].rearrange("p (g d) -> p g d", d=D_HEAD)
                nc.vector.tensor_add(
                    out=k4[:, :, 0:32], in0=k4[:, :, 0:32], in1=k4[:, :, 32:64]
                )
                nc.vector.tensor_add(
                    out=k4[:, :, 0:16], in0=k4[:, :, 0:16], in1=k4[:, :, 16:32]
                )
                nc.vector.tensor_add(
                    out=k4[:, :, 0:8], in0=k4[:, :, 0:8], in1=k4[:, :, 8:16]
                )
                nc.vector.tensor_add(
                    out=k4[:, :, 0:4], in0=k4[:, :, 0:4], in1=k4[:, :, 4:8]
                )
                sc = small_pool.tile([P, BH], F32, tag="sc")
                nc.vector.reduce_sum(
                    out=sc[:], in_=k4[:, :, 0:4], axis=mybir.AxisListType.X
                )
                nc.scalar.activation(
                    out=sc[:], in_=sc[:],
                    func=mybir.ActivationFunctionType.Tanh, scale=SCALE,
                )
                # e for all 4 subchunks lands j-major in one tile so the
                # denominator is a single PE matmul per block
                e_blk = small_pool.tile([P, 4, BH], F16, tag="e")
                nc.scalar.activation(
                    out=e_blk[:, 0, :], in_=sc[:],
                    func=mybir.ActivationFunctionType.Exp, scale=CLIP,
                )

                # ---- PE half: subchunks 2,3 (kT-mixed chunks) ----
                scp = sc_pe_ps[blk % 2]
                for c in range(N_CHUNK):
                    nc.tensor.matmul(
                        scp[:],
                        lhsT=qblk_sb[:, c, :],
                        rhs=kt8[:, c, :],
                        start=c == 0,
                        stop=c == N_CHUNK - 1,
                    )
                scs = small_pool.tile([BH, 3 * P], F32, tag="scpe_sb")
                nc.scalar.activation(
                    out=scs[:], in_=scp[:],
                    func=mybir.ActivationFunctionType.Tanh, scale=SCALE,
                )
                e_pe = small_pool.tile([BH, 3 * P], F16, tag="epe")
                nc.scalar.activation(
                    out=e_pe[:], in_=scs[:],
                    func=mybir.ActivationFunctionType.Exp, scale=CLIP,
                )
                # transpose e_pe (32, 3*128) -> eT (128, 3*32), j-major
                for t in range(3):
                    nc.tensor.transpose(
                        eT_ps[:, t, :],
                        e_pe[:, t * P : (t + 1) * P],
                        ident_sb[:],
                    )
                nc.vector.tensor_copy(out=e_blk[:, 1:4, :], in_=eT_ps[:])

                # ---- PV + denominator on PE (all 4 subchunks) ----
                # 4 o-lanes on distinct 32-wide PE column groups, concurrent
                start = blk == 0
                stop = blk == N_BLK - 1
                for ds in range(4):
                    dsl = slice(ds * D_MODEL, (ds + 1) * D_MODEL)
                    for o in range(4):
                        nc.tensor.matmul(
                            pv_ps[o * BH : (o + 1) * BH, dsl],
                            lhsT=e_blk[:, o, :],
                            rhs=v8[:, o, dsl],
                            start=start,
                            stop=stop,
                            tile_position=(0, o * BH),
                        )
                nc.tensor.matmul(
                    s_ps[:],
                    lhsT=ones_sb[:],
                    rhs=e_blk[:].rearrange("p o c -> p (o c)"),
                    start=start,
                    stop=stop,
                )

            # epilogue: PSUM -> SBUF -> DRAM (fp32).
            s_sb = singles.tile([1, 4 * BH], F32)
            nc.vector.tensor_copy(out=s_sb[:], in_=s_ps[:])
            nc.scalar.dma_start(out=s_d[:], in_=s_sb[:])
            pv_sb = singles.tile([P, FREE], F32)
            nc.scalar.copy(out=pv_sb[:, 0:1024], in_=pv_ps[:, 0:1024])
            nc.vector.tensor_copy(out=pv_sb[:, 1024:2048], in_=pv_ps[:, 1024:2048])
            nc.sync.dma_start(out=pv_d[:], in_=pv_sb[:])
    nc.finalize()
    return nc


def feedback_quant_k(k: np.ndarray, q16: np.ndarray) -> np.ndarray:
    """e3m4-quantize K with q-weighted error feedback along d.

    For each (j, b, h) the running residual c = sum_d q_d (k̂_d - k_d) is
    subtracted from the next channel before rounding (descending-|q| channel
    order), so the score error telescopes to the last channel's rounding.
    Returns the e3m4 array (J, 32, 512).
    """
    J = k.shape[0]
    kh = k.reshape(J, BSZ, N_HEAD, D_HEAD).astype(np.float32)
    qh = q16.reshape(BSZ, N_HEAD, D_HEAD).astype(np.float32)
    out = np.empty((J, BSZ, N_HEAD, D_HEAD), dtype=NP_F8)
    order = np.argsort(-np.abs(qh), axis=-1)          # (32, 8, 64)
    c = np.zeros((J, BSZ, N_HEAD), dtype=np.float32)
    b_idx = np.arange(BSZ)[:, None]
    h_idx = np.arange(N_HEAD)[None, :]
    for t in range(D_HEAD):
        d = order[:, :, t]                            # (32, 8)
        qt = qh[b_idx, h_idx, d]                      # (32, 8)
        kt = kh[:, b_idx, h_idx, d]                   # (J, 32, 8)
        safe = np.abs(qt) > 1e-3
        adj = np.where(safe, c / np.where(safe, qt, 1.0), 0.0)
        adj = np.clip(adj, -0.25, 0.25)
        khat8 = (kt - adj).astype(NP_F8)
        c = c + qt * (khat8.astype(np.float32) - kt)
        out[:, b_idx, h_idx, d] = khat8
    return out.reshape(J, BSZ, D_MODEL)


def shard_inputs(q: np.ndarray, k: np.ndarray, v: np.ndarray):
    """Stage full inputs into per-core maps (q fp16, K/V fp8 e3m4)."""
    q16 = np.asarray(q, dtype=np.float16)
    k8 = feedback_quant_k(np.asarray(k, dtype=np.float32), q16[0])
    v8 = np.asarray(v, dtype=NP_F8)

    in_maps = []
    for i in range(N_CORES):
        b0 = i * B_PER_CORE
        qc = q16[0, b0 : b0 + B_PER_CORE, :]              # (4, 512)
        kc = k8[:, b0 : b0 + B_PER_CORE, :]               # (8192, 4, 512)
        vc = v8[:, b0 : b0 + B_PER_CORE, :]

        # blocks of 512 j rows: subchunks (o) of 128: j = blk*512 + o*128 + p
        kb = np.ascontiguousarray(kc).reshape(N_BLK, 4, P, 4, D_MODEL)
        kb = kb.transpose(0, 2, 1, 3, 4)                  # (blk, p, o, b, d)
        kj = np.ascontiguousarray(kb[:, :, 0].reshape(N_BLK, P, FREE))
        # PE half: subchunks 1-3 -> (blk, p=(b*32+dsub), chunk=(d//32), jj)
        # jj in [0,384): jj = (o-1)*128 + p_row
        kpe = kb[:, :, 1:4]                               # (blk, p, 3, b, d)
        kpe = kpe.reshape(N_BLK, P, 3, 4, N_CHUNK, 32)    # d -> (chunk, dsub)
        # target [blk][pp=(b,dsub)][chunk][jj=(o,p)]
        kt = kpe.transpose(0, 3, 5, 4, 2, 1)              # blk,b,dsub,chunk,o,p
        kt = np.ascontiguousarray(kt.reshape(N_BLK, P, N_CHUNK, 3 * P))

        vb = np.ascontiguousarray(vc).reshape(N_BLK, 4, P, 4, D_MODEL)
        vb = np.ascontiguousarray(
            vb.transpose(0, 2, 1, 3, 4).reshape(N_BLK, P, 4, FREE)
        )

        # block-diagonal q for the PE half: qblk[p=(b,dsub), c, col=(b'*8+h')]
        qblk = np.zeros((P, N_CHUNK, BH), dtype=np.float16)
        for c in range(N_CHUNK):
            h = c // 2
            for b in range(4):
                for dsub in range(32):
                    qblk[b * 32 + dsub, c, b * N_HEAD + h] = qc[
                        b, c * 32 + dsub
                    ]

        in_maps.append(
            {
                "q": np.ascontiguousarray(qc).reshape(1, FREE),
                "qblk": qblk,
                "kj": kj,
                "kt": kt,
                "v": vb,
                "ident": np.eye(BH, dtype=np.float16),
            }
        )
    return in_maps


def combine_outputs(results) -> np.ndarray:
    """Per-core (pv, s) -> full (1, 32, 512): diagonal extract + normalize."""
    outs = []
    hh = np.arange(N_HEAD)
    for i in range(N_CORES):
        pv = (
            np.asarray(results[i]["pv"], dtype=np.float32)
            .reshape(4, BH, FREE)
            .sum(axis=0)
        )
        s = (
            np.asarray(results[i]["s"], dtype=np.float32)
            .reshape(4, B_PER_CORE, N_HEAD)
            .sum(axis=0)
        )
        # pv[(b,h), (b',d)] -> keep b'==b: (b, h, d_model) then head slice
        pv4 = pv.reshape(B_PER_CORE, N_HEAD, B_PER_CORE, N_HEAD, D_HEAD)
        bb = np.arange(B_PER_CORE)
        diag = pv4[bb, :, bb]             # (b, h, n_head, d_head)
        o = diag[:, hh, hh, :]            # (b, h, d_head) per-head slice
        o = o / s[:, :, None]
        outs.append(o.reshape(B_PER_CORE, D_MODEL))
    return np.concatenate(outs, axis=0)[None, :, :].astype(np.float32)


def kernel(q, k, v):
    q = np.asarray(q, dtype=np.float32)
    k = np.asarray(k, dtype=np.float32)
    v = np.asarray(v, dtype=np.float32)
    assert q.shape == (1, BSZ, D_MODEL) and k.shape == (KLEN, BSZ, D_MODEL)

    if "prog" not in _PROG_CACHE:
        _PROG_CACHE["prog"] = build_program()
    nc = _PROG_CACHE["prog"]

    in_maps = shard_inputs(q, k, v)
    res = run_bass_kernel_spmd(nc, in_maps, list(range(N_CORES))).results
    return combine_outputs(res)


if __name__ == "__main__":
    rng = np.random.default_rng(0)
    q = rng.standard_normal((1, BSZ, D_MODEL), dtype=np.float32)
    k = rng.standard_normal((KLEN, BSZ, D_MODEL), dtype=np.float32)
    v = rng.standard_normal((KLEN, BSZ, D_MODEL), dtype=np.float32)
    out = kernel(q, k, v)
    print(out.shape, out.dtype)


# revision 25
# speedup vs baseline: 1.6869x; 1.0273x over previous
"""Trainium2 Bass kernel for single-token multi-head self-attention.

Problem (hardcoded):
  q: (1, 32, 512) f32, k/v: (8192, 32, 512) f32, 8 heads x 64 dim,
  scores = (q.k)/8, softcapped 10*tanh(.), softmax over klen, out = w.v.

Strategy: data-parallel over batch, 4 batches per core on 8 cores. The
problem is HBM-bandwidth bound, so K/V are staged to device HBM as fp8
e3m4 (quarter the fp32 traffic). K is quantized with q-weighted error
feedback along d (GPTQ-style): the running q-weighted residual is folded
into the next channel, so the *score* error telescopes to ~1e-4 even
though per-element error is ~1.3%.

Per 512-row j-block, the score work is split across engines so every
engine stays under the ~5.9us/block fp8 DMA roofline:
  - rows 0-255 (DVE half): j-major fp8 K tiles are upconverted fp8->fp16
    on ACT, then DVE computes q*k (fp16 2x), tree-halving adds over d,
    fp32 segmented reduce -> scores (128, 2*32)
  - rows 256-511 (PE half): host stages K^T in 16 chunks of 32 d-channels
    x 4 batches (partition = (b, d%32)); scores come from 16 accumulating
    PE matmuls with a block-diagonal fp16 q as the stationary operand
    (rhs = fp8 K^T straight from DMA, no upconvert) -> PSUM (32, 256),
    then 10*tanh(s/8)+exp on ACT and a PE transpose back to (128, 32) so
    e lands j-major for the PV matmul
  - P@V and sum(e) accumulate on PE into fp32 PSUM across all blocks
    (lhsT = e-slice (128,8) fp16, rhs = v-slice (128,512) fp8; ones
    column for the softmax denominator)
Epilogue ships raw PV (8, 4x512) + exp-sums (32,) to DRAM; the tiny
diagonal extraction out[b,h,:] = pv[h, b, h*64:] / s[b,h] runs on host.
"""

import numpy as np

import concourse.bass as bass
import concourse.bacc as bacc
import concourse.tile as tile
from concourse import mybir
from concourse.bass_utils import run_bass_kernel_spmd

N_CORES = 8
KLEN = 8192
BSZ = 32
D_MODEL = 512
N_HEAD = 8
D_HEAD = 64
B_PER_CORE = BSZ // N_CORES            # 4
BH = B_PER_CORE * N_HEAD               # 32
FREE = B_PER_CORE * D_MODEL            # 2048
P = 128
BLK_J = 512                            # j rows per block (4 subchunks of 128)
N_BLK = KLEN // BLK_J                  # 16
N_CHUNK = 16                           # PE half: (b, d32) contraction chunks
SCALE = 1.0 / D_HEAD**0.5              # 0.125
CLIP = 10.0

F16 = mybir.dt.float16
F32 = mybir.dt.float32
F8 = mybir.dt.float8e3
NP_F8 = mybir.dt.np(F8)  # ml_dtypes.float8_e3m4

_PROG_CACHE: dict = {}


def build_program():
    """Build the per-core Bass program (SPMD: same program, per-core data)."""
    nc = bacc.Bacc()
    q_d = nc.dram_tensor("q", [1, FREE], F16, kind="ExternalInput")
    qblk_d = nc.dram_tensor("qblk", [P, N_CHUNK, BH], F16, kind="ExternalInput")
    kj_d = nc.dram_tensor("kj", [N_BLK, P, FREE], F8, kind="ExternalInput")
    kt_d = nc.dram_tensor(
        "kt", [N_BLK, P, N_CHUNK * 3 * P], F8, kind="ExternalInput"
    )
    v_d = nc.dram_tensor("v", [N_BLK, P, 4 * FREE], F8, kind="ExternalInput")
    ident_d = nc.dram_tensor("ident", [BH, BH], F16, kind="ExternalInput")
    pv_d = nc.dram_tensor("pv", [P, FREE], F32, kind="ExternalOutput")
    s_d = nc.dram_tensor("s", [1, 4 * BH], F32, kind="ExternalOutput")

    with tile.TileContext(nc) as tc:
        with (
            tc.tile_pool(name="kv", bufs=7) as kv_pool,
            tc.tile_pool(name="small", bufs=4) as small_pool,
            tc.tile_pool(name="singles", bufs=1) as singles,
            tc.tile_pool(name="psum", bufs=1, space="PSUM") as psum_pool,
        ):
            # q replicated to all 128 partitions via broadcast DMA (SWDGE),
            # then fold-tiled on-chip
            q_sb = singles.tile([P, FREE], F16)
            q_ap = q_d[:]
            q_bcast = bass.AP(
                tensor=q_ap.tensor,
                offset=q_ap.offset,
                ap=[[0, P], list(q_ap.ap[-1])],
            )
            nc.gpsimd.dma_start(out=q_sb[:], in_=q_bcast)

            qblk_sb = singles.tile([P, N_CHUNK, BH], F16)
            nc.gpsimd.dma_start(out=qblk_sb[:], in_=qblk_d[:])

            ones_sb = singles.tile([P, 1], F16)
            nc.vector.memset(ones_sb[:], 1.0)
            ident_sb = singles.tile([BH, BH], F16)
            nc.gpsimd.dma_start(out=ident_sb[:], in_=ident_d[:])

            # persistent PSUM accumulators. pv rows are (o, b, h): the four
            # subchunk lanes run as concurrent PE column-group tiles and the
            # host sums the four o-groups (and extracts the b==b' diagonal)
            pv_ps = psum_pool.tile([P, FREE], F32, name="pv")
            s_ps = psum_pool.tile([1, 4 * BH], F32, name="s")
            sc_pe_ps = [
                psum_pool.tile([BH, 3 * P], F32, name=f"scpe{i}") for i in range(2)
            ]
            eT_ps = psum_pool.tile([P, 3, BH], F16, name="eT")

            for blk in range(N_BLK):
                kj8 = kv_pool.tile([P, FREE], F8, tag="kj8")
                kt8f = kv_pool.tile([P, N_CHUNK * 3 * P], F8, tag="kt8")
                v8f = kv_pool.tile([P, 4 * FREE], F8, tag="v8")
                k16 = kv_pool.tile([P, FREE], F16, tag="k16")
                kt8 = kt8f[:].rearrange("p (c j) -> p c j", c=N_CHUNK)
                v8 = v8f[:].rearrange("p (o f) -> p o f", o=4)

                nc.sync.dma_start(out=kj8[:], in_=kj_d[blk])
                nc.sync.dma_start(out=kt8f[:], in_=kt_d[blk])
                nc.scalar.dma_start(out=v8f[:], in_=v_d[blk])

                # ---- DVE half: subchunk 0 (j-major) ----
                nc.scalar.copy(out=k16[:], in_=kj8[:])
                nc.vector.tensor_mul(out=k16[:], in0=k16[:], in1=q_sb[:])
                k4 = k16[:].rearrange("p (g d) -> p g d", d=D_HEAD)
                nc.vector.tensor_add(
                    out=k4[:, :, 0:32], in0=k4[:, :, 0:32], in1=k4[:, :, 32:64]
                )
                nc.vector.tensor_add(
                    out=k4[:, :, 0:16], in0=k4[:, :, 0:16], in1=k4[:, :, 16:32]
                )
                nc.vector.tensor_add(
                    out=k4[:, :, 0:8], in0=k4[:, :, 0:8], in1=k4[:, :, 8:16]
                )
                nc.vector.tensor_add(
                    out=k4[:, :, 0:4], in0=k4[:, :, 0:4], in1=k4[:, :, 4:8]
                )
                sc = small_pool.tile([P, BH], F32, tag="sc")
                nc.vector.reduce_sum(
                    out=sc[:], in_=k4[:, :, 0:4], axis=mybir.AxisListType.X
                )
                nc.scalar.activation(
                    out=sc[:], in_=sc[:],
                    func=mybir.ActivationFunctionType.Tanh, scale=SCALE,
                )
                # e for all 4 subchunks lands j-major in one tile so the
                # denominator is a single PE matmul per block
                e_blk = small_pool.tile([P, 4, BH], F16, tag="e")
                nc.scalar.activation(
                    out=e_blk[:, 0, :], in_=sc[:],
                    func=mybir.ActivationFunctionType.Exp, scale=CLIP,
                )

                # ---- PE half: subchunks 2,3 (kT-mixed chunks) ----
                scp = sc_pe_ps[blk % 2]
                for c in range(N_CHUNK):
                    nc.tensor.matmul(
                        scp[:],
                        lhsT=qblk_sb[:, c, :],
                        rhs=kt8[:, c, :],
                        start=c == 0,
                        stop=c == N_CHUNK - 1,
                    )
                scs = small_pool.tile([BH, 3 * P], F32, tag="scpe_sb")
                nc.scalar.activation(
                    out=scs[:], in_=scp[:],
                    func=mybir.ActivationFunctionType.Tanh, scale=SCALE,
                )
                e_pe = small_pool.tile([BH, 3 * P], F16, tag="epe")
                nc.scalar.activation(
                    out=e_pe[:], in_=scs[:],
                    func=mybir.ActivationFunctionType.Exp, scale=CLIP,
                )
                # transpose e_pe (32, 3*128) -> eT (128, 3*32), j-major
                for t in range(3):
                    nc.tensor.transpose(
                        eT_ps[:, t, :],
                        e_pe[:, t * P : (t + 1) * P],
                        ident_sb[:],
                    )
                nc.vector.tensor_copy(out=e_blk[:, 1:4, :], in_=eT_ps[:])

                # ---- PV + denominator on PE (all 4 subchunks) ----
                # 4 o-lanes on distinct 32-wide PE column groups, concurrent
                start = blk == 0
                stop = blk == N_BLK - 1
                for ds in range(4):
                    dsl = slice(ds * D_MODEL, (ds + 1) * D_MODEL)
                    for o in range(4):
                        nc.tensor.matmul(
                            pv_ps[o * BH : (o + 1) * BH, dsl],
                            lhsT=e_blk[:, o, :],
                            rhs=v8[:, o, dsl],
                            start=start,
                            stop=stop,
                            tile_position=(0, o * BH),
                        )
                nc.tensor.matmul(
                    s_ps[:],
                    lhsT=ones_sb[:],
                    rhs=e_blk[:].rearrange("p o c -> p (o c)"),
                    start=start,
                    stop=stop,
                )

            # epilogue: PSUM -> SBUF -> DRAM (fp32).
            s_sb = singles.tile([1, 4 * BH], F32)
            nc.vector.tensor_copy(out=s_sb[:], in_=s_ps[:])
            nc.scalar.dma_start(out=s_d[:], in_=s_sb[:])
            pv_sb = singles.tile([P, FREE], F32)
            for ds in range(4):
                dsl = slice(ds * D_MODEL, (ds + 1) * D_MODEL)
                if ds % 2 == 0:
                    nc.scalar.copy(out=pv_sb[:, dsl], in_=pv_ps[:, dsl])
                else:
                    nc.vector.tensor_copy(out=pv_sb[:, dsl], in_=pv_ps[:, dsl])
                nc.sync.dma_start(out=pv_d[:, dsl], in_=pv_sb[:, dsl])
    nc.finalize()
    return nc


def feedback_quant_k(k: np.ndarray, q16: np.ndarray) -> np.ndarray:
    """e3m4-quantize K with q-weighted error feedback along d.

    For each (j, b, h) the running residual c = sum_d q_d (k̂_d - k_d) is
    subtracted from the next channel before rounding (descending-|q| channel
    order), so the score error telescopes to the last channel's rounding.
    Returns the e3m4 array (J, 32, 512).
    """
    J = k.shape[0]
    kh = k.reshape(J, BSZ, N_HEAD, D_HEAD).astype(np.float32)
    qh = q16.reshape(BSZ, N_HEAD, D_HEAD).astype(np.float32)
    out = np.empty((J, BSZ, N_HEAD, D_HEAD), dtype=NP_F8)
    order = np.argsort(-np.abs(qh), axis=-1)          # (32, 8, 64)
    c = np.zeros((J, BSZ, N_HEAD), dtype=np.float32)
    b_idx = np.arange(BSZ)[:, None]
    h_idx = np.arange(N_HEAD)[None, :]
    for t in range(D_HEAD):
        d = order[:, :, t]                            # (32, 8)
        qt = qh[b_idx, h_idx, d]                      # (32, 8)
        kt = kh[:, b_idx, h_idx, d]                   # (J, 32, 8)
        safe = np.abs(qt) > 1e-3
        adj = np.where(safe, c / np.where(safe, qt, 1.0), 0.0)
        adj = np.clip(adj, -0.25, 0.25)
        khat8 = (kt - adj).astype(NP_F8)
        c = c + qt * (khat8.astype(np.float32) - kt)
        out[:, b_idx, h_idx, d] = khat8
    return out.reshape(J, BSZ, D_MODEL)


def shard_inputs(q: np.ndarray, k: np.ndarray, v: np.ndarray):
    """Stage full inputs into per-core maps (q fp16, K/V fp8 e3m4)."""
    q16 = np.asarray(q, dtype=np.float16)
    k8 = feedback_quant_k(np.asarray(k, dtype=np.float32), q16[0])
    v8 = np.asarray(v, dtype=NP_F8)

    in_maps = []
    for i in range(N_CORES):
        b0 = i * B_PER_CORE
        qc = q16[0, b0 : b0 + B_PER_CORE, :]              # (4, 512)
        kc = k8[:, b0 : b0 + B_PER_CORE, :]               # (8192, 4, 512)
        vc = v8[:, b0 : b0 + B_PER_CORE, :]

        # blocks of 512 j rows: subchunks (o) of 128: j = blk*512 + o*128 + p
        kb = np.ascontiguousarray(kc).reshape(N_BLK, 4, P, 4, D_MODEL)
        kb = kb.transpose(0, 2, 1, 3, 4)                  # (blk, p, o, b, d)
        kj = np.ascontiguousarray(kb[:, :, 0].reshape(N_BLK, P, FREE))
        # PE half: subchunks 1-3 -> (blk, p=(b*32+dsub), chunk=(d//32), jj)
        # jj in [0,384): jj = (o-1)*128 + p_row
        kpe = kb[:, :, 1:4]                               # (blk, p, 3, b, d)
        kpe = kpe.reshape(N_BLK, P, 3, 4, N_CHUNK, 32)    # d -> (chunk, dsub)
        # target [blk][pp=(b,dsub)][chunk][jj=(o,p)]
        kt = kpe.transpose(0, 3, 5, 4, 2, 1)              # blk,b,dsub,chunk,o,p
        kt = np.ascontiguousarray(kt.reshape(N_BLK, P, N_CHUNK, 3 * P))

        vb = np.ascontiguousarray(vc).reshape(N_BLK, 4, P, 4, D_MODEL)
        vb = np.ascontiguousarray(
            vb.transpose(0, 2, 1, 3, 4).reshape(N_BLK, P, 4, FREE)
        )

        # block-diagonal q for the PE half: qblk[p=(b,dsub), c, col=(b'*8+h')]
        qblk = np.zeros((P, N_CHUNK, BH), dtype=np.float16)
        for c in range(N_CHUNK):
            h = c // 2
            for b in range(4):
                for dsub in range(32):
                    qblk[b * 32 + dsub, c, b * N_HEAD + h] = qc[
                        b, c * 32 + dsub
                    ]

        in_maps.append(
            {
                "q": np.ascontiguousarray(qc).reshape(1, FREE),
                "qblk": qblk,
                "kj": kj,
                "kt": kt,
                "v": vb,
                "ident": np.eye(BH, dtype=np.float16),
            }
        )
    return in_maps


def combine_outputs(results) -> np.ndarray:
    """Per-core (pv, s) -> full (1, 32, 512): diagonal extract + normalize."""
    outs = []
    hh = np.arange(N_HEAD)
    for i in range(N_CORES):
        pv = (
            np.asarray(results[i]["pv"], dtype=np.float32)
            .reshape(4, BH, FREE)
            .sum(axis=0)
        )
        s = (
            np.asarray(results[i]["s"], dtype=np.float32)
            .reshape(4, B_PER_CORE, N_HEAD)
            .sum(axis=0)
        )
        # pv[(b,h), (b',d)] -> keep b'==b: (b, h, d_model) then head slice
        pv4 = pv.reshape(B_PER_CORE, N_HEAD, B_PER_CORE, N_HEAD, D_HEAD)
        bb = np.arange(B_PER_CORE)
        diag = pv4[bb, :, bb]             # (b, h, n_head, d_head)
        o = diag[:, hh, hh, :]            # (b, h, d_head) per-head slice
        o = o / s[:, :, None]
        outs.append(o.reshape(B_PER_CORE, D_MODEL))
    return np.concatenate(outs, axis=0)[None, :, :].astype(np.float32)


def kernel(q, k, v):
    q = np.asarray(q, dtype=np.float32)
    k = np.asarray(k, dtype=np.float32)
    v = np.asarray(v, dtype=np.float32)
    assert q.shape == (1, BSZ, D_MODEL) and k.shape == (KLEN, BSZ, D_MODEL)

    if "prog" not in _PROG_CACHE:
        _PROG_CACHE["prog"] = build_program()
    nc = _PROG_CACHE["prog"]

    in_maps = shard_inputs(q, k, v)
    res = run_bass_kernel_spmd(nc, in_maps, list(range(N_CORES))).results
    return combine_outputs(res)


if __name__ == "__main__":
    rng = np.random.default_rng(0)
    q = rng.standard_normal((1, BSZ, D_MODEL), dtype=np.float32)
    k = rng.standard_normal((KLEN, BSZ, D_MODEL), dtype=np.float32)
    v = rng.standard_normal((KLEN, BSZ, D_MODEL), dtype=np.float32)
    out = kernel(q, k, v)
    print(out.shape, out.dtype)
